# revision 67
# baseline (speedup 1.0000x reference)
"""Trainium2 Bass kernel for nn_CNNGRUforHorizon (CNN+BiGRU audio model).

Strategy: W-shard the logmel branch + fusion conv across 8 cores (each core
owns 64 of the 512 fused-map columns, with halo), replicate the tiny wave
branch, AllReduce the 64KB feature matrix, then run the 32-step BiGRU
replicated on every core. Convolutions and the GRU recurrence matmuls run
as float32r; accumulation stays fp32.

Perf structure: wave-branch matmuls are interleaved into the mel-branch
emission so the PE never idles; activations alternate between the scalar
and vector engines; the feature AllReduce runs as two 32KB halves
overlapped with the fc fusion conv, with the GRU input-gate precompute
accumulating into pre-biased gate tiles as each half lands; maxpool and
edge masks are emitted per row-group so the fusion conv starts as soon
as its rows are ready; GRU step 0 (h=0) skips its matmuls; per-step gate
math runs sigmoid/sigmoid/tanh(scale=r) on scalar with the n-gate bias
injected into PSUM by a k=1 matmul.
"""
import os
import sys

import numpy as np


def _ensure_concourse():
    try:
        import concourse  # noqa: F401
        return
    except ImportError:
        pass
    for p in ("/opt/trn_rl_repo", "/root/.axon_site/_ro/trn_rl_repo"):
        if os.path.isdir(p) and p not in sys.path:
            sys.path.insert(0, p)
    import concourse  # noqa: F401


NCORES = 8
LAST_RESULTS = None
_CACHE = {}


def _resize_matrix(n_in, n_out):
    R = np.zeros((n_in, n_out), np.float64)
    for x in range(n_out):
        c = (x + 0.5) * n_in / n_out - 0.5
        i0 = int(np.floor(c))
        w1 = c - i0
        i0c = min(max(i0, 0), n_in - 1)
        i1c = min(max(i0 + 1, 0), n_in - 1)
        R[i0c, x] += 1.0 - w1
        R[i1c, x] += w1
    return R.astype(np.float32)


def _build():
    import concourse.bacc as bacc
    import concourse.mybir as mybir
    import concourse.tile as tile

    f32 = mybir.dt.float32
    f32r = mybir.dt.float32r
    AF = mybir.ActivationFunctionType
    ALU = mybir.AluOpType
    AX = mybir.AxisListType

    nc = bacc.Bacc("TRN2", target_bir_lowering=False, debug=False,
                   num_devices=NCORES)

    def din(name, shape, dt=f32r):
        return nc.dram_tensor(name, shape, dt, kind="ExternalInput")

    lmI_d = din("lmI", [9, 8576])            # logmel im2col slice, per core
    wP_d = din("wP", [5, 3203])              # wave, stride-5 im2col planes
    Ri_d = din("Ri", [50, 66])               # resize matrix slice, per core
    msk_d = din("msk", [128, 2], f32)        # x_mel edge-col masks, per core
    zed_d = din("zed", [128, 2], f32)        # zeros (GRU h0 via warmup CC)

    w1m_d = din("w1m", [9, 64])
    bn1s_d = din("bn1s", [64, 1], f32)
    bn1b_d = din("bn1b", [64, 1], f32)
    w2m_d = din("w2m", [128, 6, 128])
    bn2s_d = din("bn2s", [128, 1], f32)
    bn2b_d = din("bn2b", [128, 1], f32)
    w3m_d = din("w3m", [128, 2, 128])
    b3_d = din("b3", [128, 2], f32)
    fcm_d = din("fcm", [128, 2, 9, 4, 128])      # fc mel-half weights
    fcw_d = din("fcw", [128, 2, 3, 3, 4, 128])   # fc wave-half (3 variants)
    fb4_d = din("fb4", [128, 4], f32)

    w1w_d = din("w1w", [5, 3, 64])
    wb1_d = din("wb1", [64, 1], f32)
    w2w_d = din("w2w", [64, 5, 128])
    wb2_d = din("wb2", [128, 1], f32)
    w3w_d = din("w3w", [128, 5, 2, 128])
    wb3w_d = din("wb3w", [128, 2], f32)
    w4wT_d = din("w4wT", [128, 2, 5, 256])
    wb4row_d = din("wb4row", [1, 256])
    one50_d = din("one50", [1, 50])

    wihT_d = din("wihT", [128, 2, 3, 4, 128])    # f32r
    brz_d = din("brz", [128, 2, 2], f32)
    bng_d = din("bng", [128, 2], f32)
    bhhnT_d = din("bhhnT", [1, 2, 128])          # f32r, for psum inject
    one2_d = din("one2", [1, 2])                 # f32r ones
    whhT_d = din("whhT", [128, 2, 3, 128])       # f32r
    clsT_d = din("clsT", [128, 2, 5], f32)
    clsb5_d = din("clsb5", [5, 1], f32)

    out_d = nc.dram_tensor("out", [1, 5], f32, kind="ExternalOutput")

    with tile.TileContext(nc) as tc:
        with (
            tc.tile_pool(name="keep", bufs=1) as keep,      # long-lived
            tc.tile_pool(name="psum", bufs=1, space="PSUM") as psp,
            tc.tile_pool(name="dram", bufs=1, space="DRAM") as dram,
            tc.tile_pool(name="sc", bufs=1) as sc,          # small scratch
        ):
            xwr8 = keep.tile([128, 2, 8, 66], f32r)
            msk = keep.tile([128, 2], f32)
            featp = keep.tile([128, 4, 32], f32)
            fb4 = keep.tile([128, 4], f32)

            def psum_aux(shape):
                return psp.tile(shape, f32, tag="aux", bufs=4, name="psaux")

            psum_mm = psum_aux

            def psum_grz(shape):
                return psp.tile(shape, f32, tag="grz", bufs=2, name="psgrz")

            def psum_gn(shape):
                return psp.tile(shape, f32, tag="gn", bufs=2, name="psgn")

            # ---------------- pools (lifetimes) -----------------------
            # right stack: wv2 (dies after resize), wv1 (dies after pool1)
            # left stack: m2 (dies at end), m1 (dies after m2), rhs (dies
            # after m1), then gru / fw / m3 allocated later.
            p_wv2 = tc.alloc_tile_pool(name="wv2", bufs=1, side="right")
            p_wv1 = tc.alloc_tile_pool(name="wv1", bufs=1, side="right")
            p_m2 = tc.alloc_tile_pool(name="mel_m2", bufs=1)
            p_m1 = tc.alloc_tile_pool(name="mel_m1", bufs=1)
            p_rhs = tc.alloc_tile_pool(name="mel_rhs", bufs=1)

            # wave tiles
            Pt = p_wv1.tile([5, 3203], f32r)
            w1w = p_wv1.tile([5, 3, 64], f32r)
            wb1 = p_wv1.tile([64, 1], f32)
            w1o = p_wv1.tile([64, 3200], f32r)
            t1 = p_wv1.tile([64, 800], f32r)
            t2 = p_wv1.tile([64, 800], f32r)

            p1t = p_wv2.tile([64, 804], f32r)
            w2w = p_wv2.tile([64, 5, 128], f32r)
            wb2 = p_wv2.tile([128, 1], f32)
            w2o = p_wv2.tile([128, 800], f32r)
            t3 = p_wv2.tile([128, 200], f32r)
            t4 = p_wv2.tile([128, 200], f32r)
            p2t = p_wv2.tile([128, 204], f32r)
            w3w = p_wv2.tile([128, 5, 2, 128], f32r)
            wb3w = p_wv2.tile([128, 2], f32)
            w3o = p_wv2.tile([128, 2, 200], f32r)
            p3t = p_wv2.tile([128, 2, 54], f32r)
            w4wT = p_wv2.tile([128, 2, 5, 256], f32r)
            wb4row = p_wv2.tile([1, 256], f32r)
            one50 = p_wv2.tile([1, 50], f32r)
            Ri = p_wv2.tile([50, 66], f32r)
            xwT = p_wv2.tile([50, 256], f32r)

            # mel tiles
            rhs1 = p_rhs.tile([9, 8576], f32r)
            m1 = p_m1.tile([128, 8848], f32r, tag="m1")
            m2 = p_m2.tile([128, 8576], f32r)
            w1m = p_m2.tile([9, 64], f32r)
            bn1s = p_m2.tile([64, 1], f32)
            bn1b = p_m2.tile([64, 1], f32)
            w2m = p_m2.tile([128, 6, 128], f32r)
            bn2s = p_m2.tile([128, 1], f32)
            bn2b = p_m2.tile([128, 1], f32)
            w3m = p_m2.tile([128, 2, 128], f32r)
            b3 = p_m2.tile([128, 2], f32)

            # ---------------- input DMAs (need-order) -----------------
            nc.sync.dma_start(Pt[:], wP_d[:])
            nc.sync.dma_start(w1w[:], w1w_d[:])
            nc.sync.dma_start(wb1[:], wb1_d[:])
            # lmI in two chunks so m1's early chunks start sooner
            nc.sync.dma_start(rhs1[:, 0:4288], lmI_d[:, 0:4288])
            nc.sync.dma_start(rhs1[:, 4288:8576], lmI_d[:, 4288:8576])
            nc.sync.dma_start(w1m[:], w1m_d[:])
            nc.sync.dma_start(bn1s[:], bn1s_d[:])
            nc.sync.dma_start(bn1b[:], bn1b_d[:])
            nc.gpsimd.dma_start(w2m[:], w2m_d[:])
            nc.gpsimd.dma_start(bn2s[:], bn2s_d[:])
            nc.gpsimd.dma_start(bn2b[:], bn2b_d[:])
            nc.gpsimd.dma_start(w2w[:], w2w_d[:])
            nc.gpsimd.dma_start(wb2[:], wb2_d[:])
            nc.gpsimd.dma_start(w3w[:], w3w_d[:])
            nc.gpsimd.dma_start(wb3w[:], wb3w_d[:])
            nc.gpsimd.dma_start(w4wT[:], w4wT_d[:])
            nc.gpsimd.dma_start(wb4row[:], wb4row_d[:])
            nc.gpsimd.dma_start(one50[:], one50_d[:])
            nc.gpsimd.dma_start(Ri[:], Ri_d[:])
            nc.gpsimd.dma_start(w3m[:], w3m_d[:])
            nc.gpsimd.dma_start(b3[:], b3_d[:])
            nc.gpsimd.dma_start(msk[:], msk_d[:])
            nc.gpsimd.dma_start(fb4[:], fb4_d[:])

            # warmup collective early (absorbs CC engine startup); its
            # output (zeros) becomes the GRU initial hidden state.
            ccz_i = dram.tile([128, 2], f32)
            ccz_o = dram.tile([128, 2], f32)
            nc.sync.dma_start(ccz_i[:], zed_d[:])
            nc.gpsimd.collective_compute(
                "AllReduce", ALU.add,
                replica_groups=[list(range(NCORES))],
                ins=[ccz_i.opt()], outs=[ccz_o.opt()])

            # ============== WAVE conv1 (replicated) ======================
            # conv1: 16000 -> 3200, k=11 s=5 via 3 taps of K=5
            for c in range(7):
                n0 = 512 * c
                n = min(512, 3200 - n0)
                ps = psum_aux([64, 512])
                for m in range(3):
                    nc.tensor.matmul(ps[:, :n], w1w[:, m, :],
                                     Pt[:, m + n0:m + n0 + n],
                                     start=(m == 0), stop=(m == 2))
                if c % 2 == 0:
                    nc.scalar.activation(w1o[:, n0:n0 + n], ps[:, :n],
                                         AF.Relu, bias=wb1[:, 0:1])
                else:
                    nc.vector.tensor_scalar(w1o[:, n0:n0 + n], ps[:, :n],
                                            wb1[:, 0:1], 0.0,
                                            op0=ALU.add, op1=ALU.max)
            # pool1 (vector) — runs while mel m1 owns the PE; split in two
            # halves so conv2's first chunk isn't gated on conv1's tail
            nc.vector.memset(p1t[:, 0:2].bitcast(f32), 0.0)
            nc.vector.memset(p1t[:, 802:804].bitcast(f32), 0.0)
            for a, b in ((0, 2560), (2560, 3200)):
                q = a // 4
                n = (b - a) // 4
                nc.vector.tensor_tensor(t1[:, q:q + n], w1o[:, a:b:4],
                                        w1o[:, a + 1:b:4], op=ALU.max)
                nc.vector.tensor_tensor(t2[:, q:q + n], w1o[:, a + 2:b:4],
                                        w1o[:, a + 3:b:4], op=ALU.max)
                nc.vector.tensor_tensor(p1t[:, 2 + q:2 + q + n],
                                        t1[:, q:q + n], t2[:, q:q + n],
                                        op=ALU.max)
            p_wv1.release()

            # ============== MEL m1 (W-sharded) ===========================
            # m1 flat [128, 8848]: partitions 0-63 hold row slot r at
            # offset 134*r (slots 0..65); partitions 64-127 hold the
            # same data shifted one row (for K=128 dy-packed m2 matmuls)
            nc.vector.memset(m1[0:64, 0:134].bitcast(f32), 0.0)
            nc.vector.memset(m1[0:64, 8710:8848].bitcast(f32), 0.0)
            nc.vector.memset(m1[64:128, 8714:8848].bitcast(f32), 0.0)
            for c in range(17):
                n0 = 512 * c
                n = min(512, 8576 - n0)
                ps = psum_mm([64, 512])
                nc.tensor.matmul(ps[:, :n], w1m[:], rhs1[:, n0:n0 + n],
                                 start=True, stop=True)
                if c % 2 == 0:
                    nc.scalar.activation(
                        m1[0:64, 134 + n0:134 + n0 + n],
                        ps[:, :n], AF.Relu,
                        bias=bn1b[:, 0:1], scale=bn1s[:, 0:1])
                else:
                    nc.vector.tensor_scalar(
                        m1[0:64, 134 + n0:134 + n0 + n],
                        ps[:, :n], bn1s[:, 0:1], bn1b[:, 0:1],
                        op0=ALU.mult, op1=ALU.add)
                    nc.vector.tensor_scalar_max(
                        m1[0:64, 134 + n0:134 + n0 + n],
                        m1[0:64, 134 + n0:134 + n0 + n], 0.0)
                eng = nc.gpsimd if c % 2 == 0 else nc.sync
                eng.dma_start(m1[64:128, n0:n0 + n],
                              m1[0:64, 134 + n0:134 + n0 + n])
            p_rhs.release()
            # ============== WAVE conv2 + MEL m2 interleaved ==============
            def wave_conv2():
                for c in range(2):
                    n0 = 512 * c
                    n = min(512, 800 - n0)
                    ps = psum_aux([128, 512])
                    for tap in range(5):
                        nc.tensor.matmul(ps[:, :n], w2w[:, tap, :],
                                         p1t[:, n0 + tap:n0 + tap + n],
                                         start=(tap == 0), stop=(tap == 4))
                    if c == 0:
                        nc.scalar.activation(w2o[:, n0:n0 + n], ps[:, :n],
                                             AF.Relu, bias=wb2[:, 0:1])
                    else:
                        nc.vector.tensor_scalar(w2o[:, n0:n0 + n],
                                                ps[:, :n], wb2[:, 0:1],
                                                0.0, op0=ALU.add,
                                                op1=ALU.max)
                nc.vector.memset(p2t[:, 0:2].bitcast(f32), 0.0)
                nc.vector.memset(p2t[:, 202:204].bitcast(f32), 0.0)
                nc.vector.tensor_tensor(t3[:], w2o[:, 0:800:4],
                                        w2o[:, 1:800:4], op=ALU.max)
                nc.vector.tensor_tensor(t4[:], w2o[:, 2:800:4],
                                        w2o[:, 3:800:4], op=ALU.max)
                nc.vector.tensor_tensor(p2t[:, 2:202], t3[:], t4[:],
                                        op=ALU.max)

            def wave_conv3():
                for oc in range(2):
                    ps = psum_aux([128, 512])
                    for tap in range(5):
                        nc.tensor.matmul(ps[:, :200], w3w[:, tap, oc, :],
                                         p2t[:, tap:tap + 200],
                                         start=(tap == 0), stop=(tap == 4))
                    nc.scalar.activation(w3o[:, oc, :], ps[:, :200],
                                         AF.Relu, bias=wb3w[:, oc:oc + 1])
                for oc in range(2):
                    nc.vector.memset(p3t[:, oc, 0:2].bitcast(f32), 0.0)
                    nc.vector.memset(p3t[:, oc, 52:54].bitcast(f32), 0.0)
                    t5 = sc.tile([128, 50], f32r, tag="t5")
                    t6 = sc.tile([128, 50], f32r, tag="t6")
                    nc.vector.tensor_tensor(t5[:], w3o[:, oc, 0:200:4],
                                            w3o[:, oc, 1:200:4], op=ALU.max)
                    nc.vector.tensor_tensor(t6[:], w3o[:, oc, 2:200:4],
                                            w3o[:, oc, 3:200:4], op=ALU.max)
                    nc.vector.tensor_tensor(p3t[:, oc, 2:52], t5[:], t6[:],
                                            op=ALU.max)

            def wave_conv4():
                ps4 = psum_aux([50, 256])
                first = True
                for ch in range(2):
                    for tap in range(5):
                        nc.tensor.matmul(ps4[:], p3t[:, ch, tap:tap + 50],
                                         w4wT[:, ch, tap, :],
                                         start=first, stop=False)
                        first = False
                nc.tensor.matmul(ps4[:], one50[:], wb4row[:],
                                 start=False, stop=True)
                nc.scalar.activation(xwT[:], ps4[:], AF.Relu)

            def wave_resize():
                # resize 50 -> local 66 cols (per-core R slice)
                for oc in range(2):
                    psR = psum_aux([128, 66])
                    nc.tensor.matmul(psR[:], xwT[:, 128 * oc:128 * (oc + 1)],
                                     Ri[:], start=True, stop=True)
                    for r in range(8):
                        nc.vector.tensor_copy(xwr8[:, oc, r, :], psR[:])

            def mel_m2(c0, c1):
                # m2: 3x3 conv, 64 -> 128 ch, flat 64x134 grid
                for c in range(c0, c1):
                    n0 = 512 * c
                    n = min(512, 8576 - n0)
                    ps = psum_mm([128, 512])
                    for j in range(6):
                        off = (268 if j >= 3 else 0) + (j % 3) + n0
                        nc.tensor.matmul(
                            ps[:, :n], w2m[:, j, :],
                            m1[:, off:off + n],
                            start=(j == 0), stop=(j == 5))
                    nc.scalar.activation(m2[:, n0:n0 + n],
                                         ps[:, :n], AF.Relu,
                                         bias=bn2b[:, 0:1],
                                         scale=bn2s[:, 0:1])

            wave_conv2()
            mel_m2(0, 4)
            wave_conv3()
            mel_m2(4, 8)
            wave_conv4()
            mel_m2(8, 12)
            wave_resize()
            mel_m2(12, 17)
            p_wv2.release()
            p_m1.release()

            # GRU weights/buffers pool (small, lives to the end)
            p_gru = tc.alloc_tile_pool(name="gru", bufs=1)
            wihT = p_gru.tile([128, 2, 3, 4, 128], f32r)
            brz = p_gru.tile([128, 2, 2], f32)
            bng = p_gru.tile([128, 2], f32)
            bhhnT = p_gru.tile([1, 2, 128], f32r)
            one2 = p_gru.tile([1, 2], f32r)
            whhT = p_gru.tile([128, 2, 3, 128], f32r)
            clsT = p_gru.tile([128, 2, 5], f32)
            clsb5 = p_gru.tile([5, 1], f32)
            ggxrz = p_gru.tile([128, 2, 32, 2], f32)
            ggxn = p_gru.tile([128, 2, 32], f32)
            ftr = p_gru.tile([128, 4, 32], f32r)
            # hall[:, d, s, 0]: hidden state per step (col 1 stays zero --
            # fp32r matmuls need an even moving-dim count)
            hall = p_gru.tile([128, 2, 33, 2], f32r)
            nc.vector.memset(hall[:].bitcast(f32), 0.0)
            hsum = p_gru.tile([128, 2], f32)
            ones32 = p_gru.tile([128, 32], f32)
            nc.vector.memset(ones32[:], 1.0)
            # gate-bias DMAs ride the (idle) gpsimd queue and the ggx
            # pre-fill runs here, long before the vector queue gets busy
            # with the m3 pools


            # ============== fc weights pool + prefetch ===================
            p_fw = tc.alloc_tile_pool(name="fcwp", bufs=1)
            xmel = p_fw.tile([128, 2, 34, 66], f32r)
            for oc in range(2):
                nc.vector.memset(xmel[:, oc, 0, :].bitcast(f32), 0.0)
                nc.vector.memset(xmel[:, oc, 33, :].bitcast(f32), 0.0)

            def fw_tiles():
                fcm = p_fw.tile([128, 2, 9, 128], f32r, tag="fcm", bufs=2,
                                name="fcm")
                fcw = p_fw.tile([128, 2, 3, 3, 128], f32r, tag="fcwt",
                                bufs=2, name="fcw")
                return fcm, fcw

            # fc oc0 weights first (needed ~30us from now); GRU weights
            # queue behind them (needed much later)
            fcm0, fcw0 = fw_tiles()
            nc.sync.dma_start(fcm0[:], fcm_d[:, :, :, 0, :])
            nc.sync.dma_start(fcw0[:], fcw_d[:, :, :, :, 0, :])
            nc.sync.dma_start(wihT[:], wihT_d[:])
            nc.sync.dma_start(brz[:], brz_d[:])
            nc.sync.dma_start(bng[:], bng_d[:])
            nc.sync.dma_start(bhhnT[:], bhhnT_d[:])
            nc.sync.dma_start(one2[:], one2_d[:])
            nc.sync.dma_start(whhT[:], whhT_d[:])
            nc.sync.dma_start(clsT[:], clsT_d[:])
            nc.sync.dma_start(clsb5[:], clsb5_d[:])

            # ============== MEL m3 + pool (chunk-interleaved) ============
            p_m3 = tc.alloc_tile_pool(name="mel_m3", bufs=1)
            m3 = p_m3.tile([128, 8576], f32r, tag="m3", bufs=1, name="m3")
            vp = p_m3.tile([128, 32, 134], f32r, tag="vp", bufs=1, name="vp")
            m3v = m3.rearrange("p (a b) -> p a b", b=134)
            for oc in range(2):
                for c in range(17):
                    n0 = 512 * c
                    n = min(512, 8576 - n0)
                    ps = psum_mm([128, 512])
                    nc.tensor.matmul(ps[:, :n], w3m[:, oc, :],
                                     m2[:, n0:n0 + n],
                                     start=True, stop=True)
                    if c % 2 == 0:
                        nc.scalar.activation(
                            m3[:, n0:n0 + n], ps[:, :n], AF.Relu,
                            bias=b3[:, oc:oc + 1])
                    else:
                        nc.vector.tensor_scalar(
                            m3[:, n0:n0 + n], ps[:, :n],
                            b3[:, oc:oc + 1], 0.0,
                            op0=ALU.add, op1=ALU.max)
                    # pool row-group g as soon as its m3 rows are done:
                    # group g covers vp rows 8g..8g+8 (m3v rows 16g..16g+16,
                    # flat cols <= 2144g+2144), ready after chunk 4g+4.
                    g = (c - 4) // 4 if c >= 4 else -1
                    if c in (4, 8, 12, 16):
                        r0 = 8 * g
                        nc.vector.tensor_tensor(
                            vp[:, r0:r0 + 8, :],
                            m3v[:, 2 * r0:2 * r0 + 16:2, :],
                            m3v[:, 2 * r0 + 1:2 * r0 + 16:2, :],
                            op=ALU.max)
                        nc.vector.tensor_tensor(
                            xmel[:, oc, 1 + r0:9 + r0, :],
                            vp[:, r0:r0 + 8, 0:132:2],
                            vp[:, r0:r0 + 8, 1:132:2], op=ALU.max)
                        # edge-col masks applied per row-group so fc rg0
                        # never waits on the last pool group
                        for j, col in ((0, 0), (1, 65)):
                            nc.vector.tensor_scalar_mul(
                                xmel[:, oc, 1 + r0:9 + r0, col:col + 1],
                                xmel[:, oc, 1 + r0:9 + r0, col:col + 1],
                                msk[:, j:j + 1])
            p_m3.release()



            # ============== FC FUSION CONV + chunked AllReduce ===========
            # gx accumulates in SBUF as each oc-chunk's AllReduce lands
            # (2-iteration lag keeps the tensor queue from stalling on the
            # collective); each psum group is a single closed matmul.
            ccins = [dram.tile([2, 128, 32], f32, name=f"ccin{i}")
                     for i in range(2)]
            ccouts = [dram.tile([2, 128, 32], f32, name=f"ccout{i}")
                      for i in range(2)]

            # ggx tiles start as pure biases; each gx chunk accumulates in
            # with vector TTs (z-gate subtracts, matching the old scale=-1)
            for d in range(2):
                nc.vector.tensor_scalar_mul(ggxrz[:, d, :, 0], ones32[:],
                                            brz[:, d, 0:1])
                nc.vector.tensor_scalar_mul(ggxrz[:, d, :, 1], ones32[:],
                                            brz[:, d, 1:2])
                nc.vector.tensor_scalar_mul(ggxn[:, d, :], ones32[:],
                                            bng[:, d:d + 1])

            def emit_gx(kk):
                gxk = psum_mm([128, 2, 3, 32])
                for d in range(2):
                    for g in range(3):
                        nc.tensor.matmul(gxk[:, d, g, :],
                                         wihT[:, d, g, kk, :],
                                         ftr[:, kk, :],
                                         start=True, stop=True)
                for d in range(2):
                    nc.vector.tensor_tensor(ggxrz[:, d, :, 0],
                                            ggxrz[:, d, :, 0],
                                            gxk[:, d, 0, :], op=ALU.add)
                    nc.vector.tensor_tensor(ggxrz[:, d, :, 1],
                                            ggxrz[:, d, :, 1],
                                            gxk[:, d, 1, :],
                                            op=ALU.subtract)
                    nc.vector.tensor_tensor(ggxn[:, d, :], ggxn[:, d, :],
                                            gxk[:, d, 2, :], op=ALU.add)

            # h0 (zeros) from the warmup collective; the scalar queue's
            # next consumer (fc xft activation) starts later than the
            # warmup finishes, so this wait costs nothing
            nc.scalar.dma_start(hall[:, :, 0:1, 0:1].bitcast(f32),
                                ccz_o[:])

            for oc in range(4):
                if oc == 0:
                    fcm, fcw = fcm0, fcw0
                else:
                    fcm, fcw = fw_tiles()
                    nc.sync.dma_start(fcm[:], fcm_d[:, :, :, oc, :])
                    nc.sync.dma_start(fcw[:], fcw_d[:, :, :, :, oc, :])
                for rg in range(4):
                    ps = psum_aux([128, 8, 64])
                    first = True
                    for ch in range(2):
                        for dy in range(3):
                            for dx in range(3):
                                nc.tensor.matmul(
                                    ps[:],
                                    fcm[:, ch, 3 * dy + dx, :],
                                    xmel[:, ch, rg * 8 + dy:
                                         rg * 8 + dy + 8, dx:dx + 64],
                                    start=first, stop=False)
                                first = False
                    for ch in range(2):
                        for dx in range(3):
                            last = (ch == 1 and dx == 2)
                            if rg == 0:
                                nc.tensor.matmul(
                                    ps[:, 0:1, :],
                                    fcw[:, ch, 1, dx, :],
                                    xwr8[:, ch, 0:1, dx:dx + 64],
                                    start=False, stop=False)
                                nc.tensor.matmul(
                                    ps[:, 1:8, :],
                                    fcw[:, ch, 0, dx, :],
                                    xwr8[:, ch, 0:7, dx:dx + 64],
                                    start=False, stop=last)
                            elif rg == 3:
                                nc.tensor.matmul(
                                    ps[:, 0:7, :],
                                    fcw[:, ch, 0, dx, :],
                                    xwr8[:, ch, 0:7, dx:dx + 64],
                                    start=False, stop=False)
                                nc.tensor.matmul(
                                    ps[:, 7:8, :],
                                    fcw[:, ch, 2, dx, :],
                                    xwr8[:, ch, 0:1, dx:dx + 64],
                                    start=False, stop=last)
                            else:
                                nc.tensor.matmul(
                                    ps[:],
                                    fcw[:, ch, 0, dx, :],
                                    xwr8[:, ch, :, dx:dx + 64],
                                    start=False, stop=last)
                    xft = sc.tile([128, 8, 64], f32r, tag="xf", bufs=3)
                    nc.scalar.activation(xft[:], ps[:], AF.Relu,
                                         bias=fb4[:, oc:oc + 1])
                    nc.vector.tensor_reduce(
                        featp[:, oc, rg * 8:rg * 8 + 8], xft[:],
                        axis=AX.X, op=ALU.add)
                # ship this oc-chunk into the staging buffer; fire an
                # AllReduce for each half (the CC engine serializes
                # collectives, so two 32KB ARs beat four 16KB ones).
                half = oc // 2
                nc.sync.dma_start(ccins[half][oc % 2], featp[:, oc, :])
                if oc % 2 == 1:
                    nc.gpsimd.collective_compute(
                        "AllReduce", ALU.add,
                        replica_groups=[list(range(NCORES))],
                        ins=[ccins[half].opt()], outs=[ccouts[half].opt()])
                    # results land directly in the f32r gx operand tile
                    # (two queues so the dmas run in parallel)
                    nc.gpsimd.dma_start(
                        ftr[:, 2 * half, :].bitcast(f32), ccouts[half][0])
                    nc.scalar.dma_start(
                        ftr[:, 2 * half + 1, :].bitcast(f32),
                        ccouts[half][1])
                # lagged gx for the first half while the second computes
                if oc == 3:
                    emit_gx(0)
                    emit_gx(1)
            emit_gx(2)
            emit_gx(3)

            # ============== GRU (replicated) =============================
            # Per (step, dir): 3 gate matmuls + a k=1 matmul injecting
            # bhh_n into the n-gate psum.  The n gate then computes as
            # tanh(r*psum + gx_n) in ONE scalar instr (scale=r), so the
            # critical chain is matmul -> sig_z' -> sig_r -> tanh -> h'
            # with r/z' in separate tiles (no false deps).
            for s in range(32):
                pss = []
                for d in range(2):
                    ps = psum_grz([128, 2, 2])
                    psn = psum_gn([128, 2])
                    nc.tensor.matmul(ps[:, 1, :], whhT[:, d, 1, :],
                                     hall[:, d, s, :],
                                     start=True, stop=True)
                    nc.tensor.matmul(ps[:, 0, :], whhT[:, d, 0, :],
                                     hall[:, d, s, :],
                                     start=True, stop=True)
                    # n-gate: bhh_n injected via k=1 matmul, then whh_n*h
                    # accumulates; only the tanh reads this tile, so the
                    # 2-matmul group fences nothing else.
                    nc.tensor.matmul(psn[:], bhhnT[0:1, d, :], one2[:],
                                     start=True, stop=False)
                    nc.tensor.matmul(psn[:], whhT[:, d, 2, :],
                                     hall[:, d, s, :],
                                     start=False, stop=True)
                    pss.append((ps, psn))
                for d in range(2):
                    t = s if d == 0 else 31 - s
                    ps, psn = pss[d]
                    zt = sc.tile([128, 1], f32, tag="zt", bufs=8)
                    nc.scalar.activation(zt[:], ps[:, 1, 0:1],
                                         AF.Sigmoid, scale=-1.0,
                                         bias=ggxrz[:, d, t, 1:2])
                    rt = sc.tile([128, 1], f32, tag="rt", bufs=8)
                    nc.scalar.activation(rt[:], ps[:, 0, 0:1],
                                         AF.Sigmoid,
                                         bias=ggxrz[:, d, t, 0:1])
                    nt = sc.tile([128, 1], f32, tag="nt", bufs=8)
                    nc.scalar.activation(nt[:], psn[:, 0:1], AF.Tanh,
                                         scale=rt[:, 0:1],
                                         bias=ggxn[:, d, t:t + 1])
                    # zt holds z' = 1-z.  hmn = h*z' - h (off critical
                    # path); h' = n*z' - hmn = (1-z)*n + z*h
                    hmn = sc.tile([128, 1], f32, tag="hmn", bufs=8)
                    nc.vector.scalar_tensor_tensor(
                        hmn[:], hall[:, d, s, 0:1], zt[:, 0:1],
                        hall[:, d, s, 0:1], op0=ALU.mult, op1=ALU.subtract)
                    nc.vector.scalar_tensor_tensor(
                        hall[:, d, s + 1, 0:1], nt[:], zt[:, 0:1], hmn[:],
                        op0=ALU.mult, op1=ALU.subtract)

            p_fw.release()
            nc.vector.tensor_reduce(hsum[:], hall[:, :, 1:33, 0],
                                    axis=AX.X, op=ALU.add)
            psc = psum_aux([5, 1])
            for d in range(2):
                nc.tensor.matmul(psc[:], clsT[:, d, :], hsum[:, d:d + 1],
                                 start=(d == 0), stop=(d == 1))
            lgt = sc.tile([5, 1], f32, tag="lgt")
            nc.scalar.activation(lgt[:], psc[:], AF.Identity,
                                 bias=clsb5[:, 0:1])
            nc.sync.dma_start(out_d[0:1, :].rearrange("a p -> p a"), lgt[:])
            p_gru.release()
            p_m2.release()

    nc.compile()
    return nc


def _prep_inputs(inputs):
    """Build the 8 per-core input maps from the full model inputs."""
    f = np.float32
    wave = np.asarray(inputs["waveform"], f).reshape(16000)
    logmel = np.asarray(inputs["logmel"], f).reshape(64, 1024)

    wp = np.zeros(16015, f)
    wp[3:16003] = wave
    wP = np.ascontiguousarray(wp.reshape(3203, 5).T)   # [5, 3203]

    R = _resize_matrix(50, 512)
    Rp = np.zeros((50, 514), f)
    Rp[:, 1:513] = R

    lmp = np.pad(logmel, ((1, 1), (4, 4)))

    w1m = np.ascontiguousarray(
        np.asarray(inputs["mc1"], f).reshape(64, 9).T)
    s1 = np.asarray(inputs["bn1g"], f) / np.sqrt(
        np.asarray(inputs["bn1v"], f) + 1e-5)
    b1 = (np.asarray(inputs["mb1"], f) - np.asarray(inputs["bn1m"], f)) * s1 \
        + np.asarray(inputs["bn1b"], f)
    mc2 = np.asarray(inputs["mc2"], f)              # [128, 64, 3, 3]
    w2m = np.zeros((128, 6, 128), f)
    for dx in range(3):
        w2m[0:64, dx, :] = mc2[:, :, 0, dx].T
        w2m[64:128, dx, :] = mc2[:, :, 1, dx].T
        w2m[0:64, 3 + dx, :] = mc2[:, :, 2, dx].T
    s2 = np.asarray(inputs["bn2g"], f) / np.sqrt(
        np.asarray(inputs["bn2v"], f) + 1e-5)
    b2 = (np.asarray(inputs["mb2"], f) - np.asarray(inputs["bn2m"], f)) * s2 \
        + np.asarray(inputs["bn2b"], f)
    w3m = np.ascontiguousarray(
        np.asarray(inputs["mc3"], f).reshape(256, 128).T.reshape(128, 2, 128))
    b3 = np.ascontiguousarray(
        np.asarray(inputs["mb3"], f).reshape(2, 128).T)

    fc = np.asarray(inputs["fc"], f)                   # [512,512,3,3]
    fcmel = fc[:, 256:, :, :]
    fcm = np.ascontiguousarray(
        fcmel.reshape(4, 128, 2, 128, 9).transpose(3, 2, 4, 0, 1))
    fcwave = fc[:, :256, :, :]
    wsum = np.stack([
        fcwave.sum(axis=2),
        fcwave[:, :, 1:, :].sum(axis=2),
        fcwave[:, :, :2, :].sum(axis=2),
    ], axis=2)                              # [512, 256, 3var, 3dx]
    fcw = np.ascontiguousarray(
        wsum.reshape(4, 128, 2, 128, 3, 3).transpose(3, 2, 4, 5, 0, 1))
    fb4 = np.ascontiguousarray(
        np.asarray(inputs["fb"], f).reshape(4, 128).T)

    wc1 = np.asarray(inputs["wc1"], f).reshape(64, 11)
    w1w = np.zeros((5, 3, 64), f)
    for tap in range(11):
        w1w[tap % 5, tap // 5, :] = wc1[:, tap]
    w2w = np.ascontiguousarray(
        np.asarray(inputs["wc2"], f).reshape(128, 64, 5)
        .transpose(1, 2, 0))
    w3w = np.ascontiguousarray(
        np.asarray(inputs["wc3"], f).reshape(256, 128, 5)
        .transpose(1, 2, 0).reshape(128, 5, 2, 128))
    wb3w = np.ascontiguousarray(
        np.asarray(inputs["wb3"], f).reshape(2, 128).T)
    w4wT = np.ascontiguousarray(
        np.asarray(inputs["wc4"], f).reshape(256, 256, 5)
        .transpose(1, 2, 0).reshape(2, 128, 5, 256).transpose(1, 0, 2, 3))
    wb4row = np.asarray(inputs["wb4"], f).reshape(1, 256)
    one50 = np.ones((1, 50), f)

    def gru_prep(d):
        wih = np.asarray(inputs[f"wih_{d}"], f) / 512.0
        whh = np.asarray(inputs[f"whh_{d}"], f)
        bih = np.asarray(inputs[f"bih_{d}"], f)
        bhh = np.asarray(inputs[f"bhh_{d}"], f)
        wihT = np.ascontiguousarray(
            wih.reshape(3, 128, 4, 128).transpose(3, 0, 2, 1))
        whhT = np.ascontiguousarray(
            whh.reshape(3, 128, 128).transpose(2, 0, 1))
        brz = (bih + bhh)[:256].reshape(2, 128).T
        return wihT, whhT, brz, bih[256:], bhh[256:]

    wihT_f, whhT_f, brz_f, bn_f, bhn_f = gru_prep("f")
    wihT_b, whhT_b, brz_b, bn_b, bhn_b = gru_prep("b")
    wihT = np.ascontiguousarray(np.stack([wihT_f, wihT_b], axis=1))
    whhT = np.ascontiguousarray(np.stack([whhT_f, whhT_b], axis=1))
    brz = np.ascontiguousarray(np.stack([brz_f, brz_b], axis=1))
    brz[:, :, 1] *= -1.0
    bng = np.ascontiguousarray(np.stack([bn_f, bn_b], axis=1))
    bhhn = np.ascontiguousarray(np.stack([bhn_f, bhn_b], axis=1))
    bhhnT = np.ascontiguousarray(bhhn.T.reshape(1, 2, 128))
    clsW = np.asarray(inputs["clsW"], f) / 32.0
    clsT = np.ascontiguousarray(
        clsW.reshape(5, 2, 128).transpose(2, 1, 0))
    clsb5 = np.asarray(inputs["clsb"], f).reshape(5, 1)

    shared = dict(
        wP=wP, w1m=w1m, bn1s=s1.reshape(64, 1), bn1b=b1.reshape(64, 1),
        w2m=w2m, bn2s=s2.reshape(128, 1), bn2b=b2.reshape(128, 1),
        w3m=w3m, b3=b3, fcm=fcm, fcw=fcw, fb4=fb4,
        w1w=w1w, wb1=np.asarray(inputs["wb1"], f).reshape(64, 1),
        w2w=w2w, wb2=np.asarray(inputs["wb2"], f).reshape(128, 1),
        w3w=w3w, wb3w=wb3w, w4wT=w4wT, wb4row=wb4row, one50=one50,
        wihT=wihT, brz=brz, bng=bng, bhhnT=bhhnT, whhT=whhT,
        one2=np.ones((1, 2), f),
        clsT=clsT, clsb5=clsb5, zed=np.zeros((128, 2), f),
    )
    in_maps = []
    for i in range(NCORES):
        m = dict(shared)
        lms = lmp[:, 128 * i:128 * i + 136]
        lmI = np.empty((9, 8576), f)
        for dy in range(3):
            for dx in range(3):
                lmI[3 * dy + dx] = lms[dy:dy + 64, dx:dx + 134].reshape(-1)
        m["lmI"] = lmI
        m["Ri"] = np.ascontiguousarray(Rp[:, 64 * i:64 * i + 66])
        mk = np.ones((128, 2), f)
        if i == 0:
            mk[:, 0] = 0.0
        if i == NCORES - 1:
            mk[:, 1] = 0.0
        m["msk"] = mk
        in_maps.append(m)
    return in_maps


def kernel(**inputs):
    global LAST_RESULTS
    _ensure_concourse()
    from concourse import bass_utils

    if "nc" not in _CACHE:
        _CACHE["nc"] = _build()
    nc = _CACHE["nc"]
    in_maps = _prep_inputs(inputs)
    res = bass_utils.run_bass_kernel_spmd(
        nc, in_maps, core_ids=list(range(NCORES)))
    LAST_RESULTS = res
    return res.results[0]["out"]


if __name__ == "__main__":
    _ensure_concourse()
    _build()
    print("build + compile OK")


# revision 68
# speedup vs baseline: 1.1801x; 1.1801x over previous
"""Trainium2 Bass kernel for nn_CNNGRUforHorizon (CNN+BiGRU audio model).

Strategy: W-shard the logmel branch + fusion conv across 8 cores (each core
owns 64 of the 512 fused-map columns, with halo), replicate the tiny wave
branch, AllReduce the 64KB feature matrix, then run the 32-step BiGRU
replicated on every core. Convolutions and the GRU recurrence matmuls run
as float32r; accumulation stays fp32.

Perf structure: wave-branch matmuls are interleaved into the mel-branch
emission so the PE never idles; activations alternate between the scalar
and vector engines; the feature AllReduce runs as two 32KB halves
overlapped with the fc fusion conv, with the GRU input-gate precompute
accumulating into pre-biased gate tiles as each half lands; maxpool and
edge masks are emitted per row-group so the fusion conv starts as soon
as its rows are ready; GRU step 0 (h=0) skips its matmuls; per-step gate
math runs sigmoid/sigmoid/tanh(scale=r) on scalar with the n-gate bias
injected into PSUM by a k=1 matmul.
"""
import os
import sys

import numpy as np


def _ensure_concourse():
    try:
        import concourse  # noqa: F401
        return
    except ImportError:
        pass
    for p in ("/opt/trn_rl_repo", "/root/.axon_site/_ro/trn_rl_repo"):
        if os.path.isdir(p) and p not in sys.path:
            sys.path.insert(0, p)
    import concourse  # noqa: F401


NCORES = 8
LAST_RESULTS = None
_CACHE = {}


def _resize_matrix(n_in, n_out):
    R = np.zeros((n_in, n_out), np.float64)
    for x in range(n_out):
        c = (x + 0.5) * n_in / n_out - 0.5
        i0 = int(np.floor(c))
        w1 = c - i0
        i0c = min(max(i0, 0), n_in - 1)
        i1c = min(max(i0 + 1, 0), n_in - 1)
        R[i0c, x] += 1.0 - w1
        R[i1c, x] += w1
    return R.astype(np.float32)


def _build():
    import concourse.bacc as bacc
    import concourse.mybir as mybir
    import concourse.tile as tile

    f32 = mybir.dt.float32
    f32r = mybir.dt.float32r
    AF = mybir.ActivationFunctionType
    ALU = mybir.AluOpType
    AX = mybir.AxisListType

    nc = bacc.Bacc("TRN2", target_bir_lowering=False, debug=False,
                   num_devices=NCORES)

    def din(name, shape, dt=f32r):
        return nc.dram_tensor(name, shape, dt, kind="ExternalInput")

    lmI_d = din("lmI", [9, 8576])            # logmel im2col slice, per core
    wP_d = din("wP", [5, 3203])              # wave, stride-5 im2col planes
    Ri_d = din("Ri", [50, 66])               # resize matrix slice, per core
    msk_d = din("msk", [128, 2], f32)        # x_mel edge-col masks, per core
    zed_d = din("zed", [128, 2], f32)        # zeros (GRU h0 via warmup CC)

    w1m_d = din("w1m", [9, 64])
    bn1s_d = din("bn1s", [64, 1], f32)
    bn1b_d = din("bn1b", [64, 1], f32)
    w2m_d = din("w2m", [128, 6, 128])
    bn2s_d = din("bn2s", [128, 1], f32)
    bn2b_d = din("bn2b", [128, 1], f32)
    w3m_d = din("w3m", [128, 2, 128])
    b3_d = din("b3", [128, 2], f32)
    fcm_d = din("fcm", [128, 2, 9, 4, 128])      # fc mel-half weights
    fcw_d = din("fcw", [128, 2, 3, 3, 4, 128])   # fc wave-half (3 variants)
    fb4_d = din("fb4", [128, 4], f32)

    w1w_d = din("w1w", [5, 3, 64])
    wb1_d = din("wb1", [64, 1], f32)
    w2w_d = din("w2w", [64, 5, 128])
    wb2_d = din("wb2", [128, 1], f32)
    w3w_d = din("w3w", [128, 5, 2, 128])
    wb3w_d = din("wb3w", [128, 2], f32)
    w4wT_d = din("w4wT", [128, 2, 5, 256])
    wb4row_d = din("wb4row", [1, 256])
    one50_d = din("one50", [1, 50])

    wihT_d = din("wihT", [128, 2, 3, 4, 128])    # f32r
    brz_d = din("brz", [128, 2, 2], f32)
    bng_d = din("bng", [128, 2], f32)
    bhhnT_d = din("bhhnT", [1, 2, 128])          # f32r, for psum inject
    one2_d = din("one2", [1, 2])                 # f32r ones
    whhT_d = din("whhT", [128, 2, 3, 128])       # f32r
    clsT_d = din("clsT", [128, 2, 5], f32)
    clsb5_d = din("clsb5", [5, 1], f32)

    out_d = nc.dram_tensor("out", [1, 5], f32, kind="ExternalOutput")

    with tile.TileContext(nc) as tc:
        with (
            tc.tile_pool(name="keep", bufs=1) as keep,      # long-lived
            tc.tile_pool(name="psum", bufs=1, space="PSUM") as psp,
            tc.tile_pool(name="dram", bufs=1, space="DRAM") as dram,
            tc.tile_pool(name="sc", bufs=1) as sc,          # small scratch
        ):
            xwr8 = keep.tile([128, 2, 8, 66], f32r)
            msk = keep.tile([128, 2], f32)
            featp = keep.tile([128, 4, 32], f32)
            fb4 = keep.tile([128, 4], f32)

            def psum_aux(shape):
                return psp.tile(shape, f32, tag="aux", bufs=4, name="psaux")

            psum_mm = psum_aux

            def psum_grz(shape):
                return psp.tile(shape, f32, tag="grz", bufs=2, name="psgrz")

            def psum_gn(shape):
                return psp.tile(shape, f32, tag="gn", bufs=2, name="psgn")

            # ---------------- pools (lifetimes) -----------------------
            # right stack: wv2 (dies after resize), wv1 (dies after pool1)
            # left stack: m2 (dies at end), m1 (dies after m2), rhs (dies
            # after m1), then gru / fw / m3 allocated later.
            p_wv2 = tc.alloc_tile_pool(name="wv2", bufs=1, side="right")
            p_wv1 = tc.alloc_tile_pool(name="wv1", bufs=1, side="right")
            p_m2 = tc.alloc_tile_pool(name="mel_m2", bufs=1)
            p_m1 = tc.alloc_tile_pool(name="mel_m1", bufs=1)
            p_rhs = tc.alloc_tile_pool(name="mel_rhs", bufs=1)

            # wave tiles
            Pt = p_wv1.tile([5, 3203], f32r)
            w1w = p_wv1.tile([5, 3, 64], f32r)
            wb1 = p_wv1.tile([64, 1], f32)
            w1o = p_wv1.tile([64, 3200], f32r)
            t1 = p_wv1.tile([64, 800], f32r)
            t2 = p_wv1.tile([64, 800], f32r)

            p1t = p_wv2.tile([64, 804], f32r)
            w2w = p_wv2.tile([64, 5, 128], f32r)
            wb2 = p_wv2.tile([128, 1], f32)
            w2o = p_wv2.tile([128, 800], f32r)
            t3 = p_wv2.tile([128, 200], f32r)
            t4 = p_wv2.tile([128, 200], f32r)
            p2t = p_wv2.tile([128, 204], f32r)
            w3w = p_wv2.tile([128, 5, 2, 128], f32r)
            wb3w = p_wv2.tile([128, 2], f32)
            w3o = p_wv2.tile([128, 2, 200], f32r)
            p3t = p_wv2.tile([128, 2, 54], f32r)
            w4wT = p_wv2.tile([128, 2, 5, 256], f32r)
            wb4row = p_wv2.tile([1, 256], f32r)
            one50 = p_wv2.tile([1, 50], f32r)
            Ri = p_wv2.tile([50, 66], f32r)
            xwT = p_wv2.tile([50, 256], f32r)

            # mel tiles
            rhs1 = p_rhs.tile([9, 8576], f32r)
            m1 = p_m1.tile([128, 8848], f32r, tag="m1")
            m2 = p_m2.tile([128, 8576], f32r)
            w1m = p_m2.tile([9, 64], f32r)
            bn1s = p_m2.tile([64, 1], f32)
            bn1b = p_m2.tile([64, 1], f32)
            w2m = p_m2.tile([128, 6, 128], f32r)
            bn2s = p_m2.tile([128, 1], f32)
            bn2b = p_m2.tile([128, 1], f32)
            w3m = p_m2.tile([128, 2, 128], f32r)
            b3 = p_m2.tile([128, 2], f32)

            # ---------------- input DMAs (need-order) -----------------
            nc.sync.dma_start(Pt[:], wP_d[:])
            nc.sync.dma_start(w1w[:], w1w_d[:])
            nc.sync.dma_start(wb1[:], wb1_d[:])
            # lmI in two chunks so m1's early chunks start sooner
            nc.sync.dma_start(rhs1[:, 0:4288], lmI_d[:, 0:4288])
            nc.sync.dma_start(rhs1[:, 4288:8576], lmI_d[:, 4288:8576])
            nc.sync.dma_start(w1m[:], w1m_d[:])
            nc.sync.dma_start(bn1s[:], bn1s_d[:])
            nc.sync.dma_start(bn1b[:], bn1b_d[:])
            nc.gpsimd.dma_start(w2m[:], w2m_d[:])
            nc.gpsimd.dma_start(bn2s[:], bn2s_d[:])
            nc.gpsimd.dma_start(bn2b[:], bn2b_d[:])
            nc.gpsimd.dma_start(w2w[:], w2w_d[:])
            nc.gpsimd.dma_start(wb2[:], wb2_d[:])
            nc.gpsimd.dma_start(w3w[:], w3w_d[:])
            nc.gpsimd.dma_start(wb3w[:], wb3w_d[:])
            nc.gpsimd.dma_start(w4wT[:], w4wT_d[:])
            nc.gpsimd.dma_start(wb4row[:], wb4row_d[:])
            nc.gpsimd.dma_start(one50[:], one50_d[:])
            nc.gpsimd.dma_start(Ri[:], Ri_d[:])
            nc.gpsimd.dma_start(w3m[:], w3m_d[:])
            nc.gpsimd.dma_start(b3[:], b3_d[:])
            nc.gpsimd.dma_start(msk[:], msk_d[:])
            nc.gpsimd.dma_start(fb4[:], fb4_d[:])

            # PE clock warm-up: ~3us of dummy matmuls on memset data run
            # while the first input DMAs are still in flight, so the real
            # conv matmuls start at full p-state instead of ~2x slow
            wmup = p_wv1.tile([64, 512], f32r)
            nc.vector.memset(wmup[:].bitcast(f32), 0.0)
            psw = psum_aux([64, 512])
            for i in range(8):
                nc.tensor.matmul(psw[:], wmup[:, 0:64], wmup[:],
                                 start=(i == 0), stop=(i == 7))
            wjunk = sc.tile([64, 512], f32, tag="wj")
            nc.vector.tensor_copy(wjunk[:], psw[:])

            # warmup collective early (absorbs CC engine startup); its
            # output (zeros) becomes the GRU initial hidden state.
            ccz_i = dram.tile([128, 2], f32)
            ccz_o = dram.tile([128, 2], f32)
            nc.sync.dma_start(ccz_i[:], zed_d[:])
            nc.gpsimd.collective_compute(
                "AllReduce", ALU.add,
                replica_groups=[list(range(NCORES))],
                ins=[ccz_i.opt()], outs=[ccz_o.opt()])

            # ============== WAVE conv1 (replicated) ======================
            # conv1: 16000 -> 3200, k=11 s=5 via 3 taps of K=5
            for c in range(7):
                n0 = 512 * c
                n = min(512, 3200 - n0)
                ps = psum_aux([64, 512])
                for m in range(3):
                    nc.tensor.matmul(ps[:, :n], w1w[:, m, :],
                                     Pt[:, m + n0:m + n0 + n],
                                     start=(m == 0), stop=(m == 2))
                if c % 2 == 0:
                    nc.scalar.activation(w1o[:, n0:n0 + n], ps[:, :n],
                                         AF.Relu, bias=wb1[:, 0:1])
                else:
                    nc.vector.tensor_scalar(w1o[:, n0:n0 + n], ps[:, :n],
                                            wb1[:, 0:1], 0.0,
                                            op0=ALU.add, op1=ALU.max)
            # pool1 (vector) — runs while mel m1 owns the PE; split in two
            # halves so conv2's first chunk isn't gated on conv1's tail
            nc.vector.memset(p1t[:, 0:2].bitcast(f32), 0.0)
            nc.vector.memset(p1t[:, 802:804].bitcast(f32), 0.0)
            for a, b in ((0, 2560), (2560, 3200)):
                q = a // 4
                n = (b - a) // 4
                nc.vector.tensor_tensor(t1[:, q:q + n], w1o[:, a:b:4],
                                        w1o[:, a + 1:b:4], op=ALU.max)
                nc.vector.tensor_tensor(t2[:, q:q + n], w1o[:, a + 2:b:4],
                                        w1o[:, a + 3:b:4], op=ALU.max)
                nc.vector.tensor_tensor(p1t[:, 2 + q:2 + q + n],
                                        t1[:, q:q + n], t2[:, q:q + n],
                                        op=ALU.max)
            p_wv1.release()

            # ============== MEL m1 (W-sharded) ===========================
            # m1 flat [128, 8848]: partitions 0-63 hold row slot r at
            # offset 134*r (slots 0..65); partitions 64-127 hold the
            # same data shifted one row (for K=128 dy-packed m2 matmuls)
            nc.vector.memset(m1[0:64, 0:134].bitcast(f32), 0.0)
            nc.vector.memset(m1[0:64, 8710:8848].bitcast(f32), 0.0)
            nc.vector.memset(m1[64:128, 8714:8848].bitcast(f32), 0.0)
            for c in range(17):
                n0 = 512 * c
                n = min(512, 8576 - n0)
                ps = psum_mm([64, 512])
                nc.tensor.matmul(ps[:, :n], w1m[:], rhs1[:, n0:n0 + n],
                                 start=True, stop=True)
                if c % 2 == 0:
                    nc.scalar.activation(
                        m1[0:64, 134 + n0:134 + n0 + n],
                        ps[:, :n], AF.Relu,
                        bias=bn1b[:, 0:1], scale=bn1s[:, 0:1])
                else:
                    nc.vector.tensor_scalar(
                        m1[0:64, 134 + n0:134 + n0 + n],
                        ps[:, :n], bn1s[:, 0:1], bn1b[:, 0:1],
                        op0=ALU.mult, op1=ALU.add)
                    nc.vector.tensor_scalar_max(
                        m1[0:64, 134 + n0:134 + n0 + n],
                        m1[0:64, 134 + n0:134 + n0 + n], 0.0)
                eng = nc.gpsimd if c % 2 == 0 else nc.sync
                eng.dma_start(m1[64:128, n0:n0 + n],
                              m1[0:64, 134 + n0:134 + n0 + n])
            p_rhs.release()
            # ============== WAVE conv2 + MEL m2 interleaved ==============
            def wave_conv2():
                for c in range(2):
                    n0 = 512 * c
                    n = min(512, 800 - n0)
                    ps = psum_aux([128, 512])
                    for tap in range(5):
                        nc.tensor.matmul(ps[:, :n], w2w[:, tap, :],
                                         p1t[:, n0 + tap:n0 + tap + n],
                                         start=(tap == 0), stop=(tap == 4))
                    if c == 0:
                        nc.scalar.activation(w2o[:, n0:n0 + n], ps[:, :n],
                                             AF.Relu, bias=wb2[:, 0:1])
                    else:
                        nc.vector.tensor_scalar(w2o[:, n0:n0 + n],
                                                ps[:, :n], wb2[:, 0:1],
                                                0.0, op0=ALU.add,
                                                op1=ALU.max)
                nc.vector.memset(p2t[:, 0:2].bitcast(f32), 0.0)
                nc.vector.memset(p2t[:, 202:204].bitcast(f32), 0.0)
                nc.vector.tensor_tensor(t3[:], w2o[:, 0:800:4],
                                        w2o[:, 1:800:4], op=ALU.max)
                nc.vector.tensor_tensor(t4[:], w2o[:, 2:800:4],
                                        w2o[:, 3:800:4], op=ALU.max)
                nc.vector.tensor_tensor(p2t[:, 2:202], t3[:], t4[:],
                                        op=ALU.max)

            def wave_conv3():
                for oc in range(2):
                    ps = psum_aux([128, 512])
                    for tap in range(5):
                        nc.tensor.matmul(ps[:, :200], w3w[:, tap, oc, :],
                                         p2t[:, tap:tap + 200],
                                         start=(tap == 0), stop=(tap == 4))
                    nc.scalar.activation(w3o[:, oc, :], ps[:, :200],
                                         AF.Relu, bias=wb3w[:, oc:oc + 1])
                for oc in range(2):
                    nc.vector.memset(p3t[:, oc, 0:2].bitcast(f32), 0.0)
                    nc.vector.memset(p3t[:, oc, 52:54].bitcast(f32), 0.0)
                    t5 = sc.tile([128, 50], f32r, tag="t5")
                    t6 = sc.tile([128, 50], f32r, tag="t6")
                    nc.vector.tensor_tensor(t5[:], w3o[:, oc, 0:200:4],
                                            w3o[:, oc, 1:200:4], op=ALU.max)
                    nc.vector.tensor_tensor(t6[:], w3o[:, oc, 2:200:4],
                                            w3o[:, oc, 3:200:4], op=ALU.max)
                    nc.vector.tensor_tensor(p3t[:, oc, 2:52], t5[:], t6[:],
                                            op=ALU.max)

            def wave_conv4():
                ps4 = psum_aux([50, 256])
                first = True
                for ch in range(2):
                    for tap in range(5):
                        nc.tensor.matmul(ps4[:], p3t[:, ch, tap:tap + 50],
                                         w4wT[:, ch, tap, :],
                                         start=first, stop=False)
                        first = False
                nc.tensor.matmul(ps4[:], one50[:], wb4row[:],
                                 start=False, stop=True)
                nc.scalar.activation(xwT[:], ps4[:], AF.Relu)

            def wave_resize():
                # resize 50 -> local 66 cols (per-core R slice)
                for oc in range(2):
                    psR = psum_aux([128, 66])
                    nc.tensor.matmul(psR[:], xwT[:, 128 * oc:128 * (oc + 1)],
                                     Ri[:], start=True, stop=True)
                    for r in range(8):
                        nc.vector.tensor_copy(xwr8[:, oc, r, :], psR[:])

            def mel_m2(c0, c1):
                # m2: 3x3 conv, 64 -> 128 ch, flat 64x134 grid
                for c in range(c0, c1):
                    n0 = 512 * c
                    n = min(512, 8576 - n0)
                    ps = psum_mm([128, 512])
                    for j in range(6):
                        off = (268 if j >= 3 else 0) + (j % 3) + n0
                        nc.tensor.matmul(
                            ps[:, :n], w2m[:, j, :],
                            m1[:, off:off + n],
                            start=(j == 0), stop=(j == 5))
                    nc.scalar.activation(m2[:, n0:n0 + n],
                                         ps[:, :n], AF.Relu,
                                         bias=bn2b[:, 0:1],
                                         scale=bn2s[:, 0:1])

            wave_conv2()
            mel_m2(0, 4)
            wave_conv3()
            mel_m2(4, 8)
            wave_conv4()
            mel_m2(8, 12)
            wave_resize()
            mel_m2(12, 17)
            p_wv2.release()
            p_m1.release()

            # GRU weights/buffers pool (small, lives to the end)
            p_gru = tc.alloc_tile_pool(name="gru", bufs=1)
            wihT = p_gru.tile([128, 2, 3, 4, 128], f32r)
            brz = p_gru.tile([128, 2, 2], f32)
            bng = p_gru.tile([128, 2], f32)
            bhhnT = p_gru.tile([1, 2, 128], f32r)
            one2 = p_gru.tile([1, 2], f32r)
            whhT = p_gru.tile([128, 2, 3, 128], f32r)
            clsT = p_gru.tile([128, 2, 5], f32)
            clsb5 = p_gru.tile([5, 1], f32)
            ggxrz = p_gru.tile([128, 2, 32, 2], f32)
            ggxn = p_gru.tile([128, 2, 32], f32)
            ftr = p_gru.tile([128, 4, 32], f32r)
            # hall[:, d, s, 0]: hidden state per step (col 1 stays zero --
            # fp32r matmuls need an even moving-dim count)
            hall = p_gru.tile([128, 2, 33, 2], f32r)
            nc.vector.memset(hall[:].bitcast(f32), 0.0)
            hsum = p_gru.tile([128, 2], f32)
            ones32 = p_gru.tile([128, 32], f32)
            nc.vector.memset(ones32[:], 1.0)
            # gate-bias DMAs ride the (idle) gpsimd queue and the ggx
            # pre-fill runs here, long before the vector queue gets busy
            # with the m3 pools


            # ============== fc weights pool + prefetch ===================
            p_fw = tc.alloc_tile_pool(name="fcwp", bufs=1)
            xmel = p_fw.tile([128, 2, 34, 66], f32r)
            for oc in range(2):
                nc.vector.memset(xmel[:, oc, 0, :].bitcast(f32), 0.0)
                nc.vector.memset(xmel[:, oc, 33, :].bitcast(f32), 0.0)

            def fw_tiles():
                fcm = p_fw.tile([128, 2, 9, 128], f32r, tag="fcm", bufs=2,
                                name="fcm")
                fcw = p_fw.tile([128, 2, 3, 3, 128], f32r, tag="fcwt",
                                bufs=2, name="fcw")
                return fcm, fcw

            # fc oc0 weights first (needed ~30us from now); GRU weights
            # queue behind them (needed much later)
            fcm0, fcw0 = fw_tiles()
            nc.sync.dma_start(fcm0[:], fcm_d[:, :, :, 0, :])
            nc.sync.dma_start(fcw0[:], fcw_d[:, :, :, :, 0, :])
            nc.sync.dma_start(wihT[:], wihT_d[:])
            nc.sync.dma_start(brz[:], brz_d[:])
            nc.sync.dma_start(bng[:], bng_d[:])
            nc.sync.dma_start(bhhnT[:], bhhnT_d[:])
            nc.sync.dma_start(one2[:], one2_d[:])
            nc.sync.dma_start(whhT[:], whhT_d[:])
            nc.sync.dma_start(clsT[:], clsT_d[:])
            nc.sync.dma_start(clsb5[:], clsb5_d[:])

            # ============== MEL m3 + pool (chunk-interleaved) ============
            p_m3 = tc.alloc_tile_pool(name="mel_m3", bufs=1)
            m3 = p_m3.tile([128, 8576], f32r, tag="m3", bufs=1, name="m3")
            vp = p_m3.tile([128, 32, 134], f32r, tag="vp", bufs=1, name="vp")
            m3v = m3.rearrange("p (a b) -> p a b", b=134)
            for oc in range(2):
                for c in range(17):
                    n0 = 512 * c
                    n = min(512, 8576 - n0)
                    ps = psum_mm([128, 512])
                    nc.tensor.matmul(ps[:, :n], w3m[:, oc, :],
                                     m2[:, n0:n0 + n],
                                     start=True, stop=True)
                    if c % 2 == 0:
                        nc.scalar.activation(
                            m3[:, n0:n0 + n], ps[:, :n], AF.Relu,
                            bias=b3[:, oc:oc + 1])
                    else:
                        nc.vector.tensor_scalar(
                            m3[:, n0:n0 + n], ps[:, :n],
                            b3[:, oc:oc + 1], 0.0,
                            op0=ALU.add, op1=ALU.max)
                    # pool row-group g as soon as its m3 rows are done:
                    # group g covers vp rows 8g..8g+8 (m3v rows 16g..16g+16,
                    # flat cols <= 2144g+2144), ready after chunk 4g+4.
                    g = (c - 4) // 4 if c >= 4 else -1
                    if c in (4, 8, 12, 16):
                        r0 = 8 * g
                        nc.vector.tensor_tensor(
                            vp[:, r0:r0 + 8, :],
                            m3v[:, 2 * r0:2 * r0 + 16:2, :],
                            m3v[:, 2 * r0 + 1:2 * r0 + 16:2, :],
                            op=ALU.max)
                        nc.vector.tensor_tensor(
                            xmel[:, oc, 1 + r0:9 + r0, :],
                            vp[:, r0:r0 + 8, 0:132:2],
                            vp[:, r0:r0 + 8, 1:132:2], op=ALU.max)
                        # edge-col masks applied per row-group so fc rg0
                        # never waits on the last pool group
                        for j, col in ((0, 0), (1, 65)):
                            nc.vector.tensor_scalar_mul(
                                xmel[:, oc, 1 + r0:9 + r0, col:col + 1],
                                xmel[:, oc, 1 + r0:9 + r0, col:col + 1],
                                msk[:, j:j + 1])
            p_m3.release()



            # ============== FC FUSION CONV + chunked AllReduce ===========
            # gx accumulates in SBUF as each oc-chunk's AllReduce lands
            # (2-iteration lag keeps the tensor queue from stalling on the
            # collective); each psum group is a single closed matmul.
            ccins = [dram.tile([2, 128, 32], f32, name=f"ccin{i}")
                     for i in range(2)]
            ccouts = [dram.tile([2, 128, 32], f32, name=f"ccout{i}")
                      for i in range(2)]

            # ggx tiles start as pure biases; each gx chunk accumulates in
            # with vector TTs (z-gate subtracts, matching the old scale=-1)
            for d in range(2):
                nc.vector.tensor_scalar_mul(ggxrz[:, d, :, 0], ones32[:],
                                            brz[:, d, 0:1])
                nc.vector.tensor_scalar_mul(ggxrz[:, d, :, 1], ones32[:],
                                            brz[:, d, 1:2])
                nc.vector.tensor_scalar_mul(ggxn[:, d, :], ones32[:],
                                            bng[:, d:d + 1])

            def emit_gx(kk):
                gxk = psum_mm([128, 2, 3, 32])
                for d in range(2):
                    for g in range(3):
                        nc.tensor.matmul(gxk[:, d, g, :],
                                         wihT[:, d, g, kk, :],
                                         ftr[:, kk, :],
                                         start=True, stop=True)
                for d in range(2):
                    nc.vector.tensor_tensor(ggxrz[:, d, :, 0],
                                            ggxrz[:, d, :, 0],
                                            gxk[:, d, 0, :], op=ALU.add)
                    nc.vector.tensor_tensor(ggxrz[:, d, :, 1],
                                            ggxrz[:, d, :, 1],
                                            gxk[:, d, 1, :],
                                            op=ALU.subtract)
                    nc.vector.tensor_tensor(ggxn[:, d, :], ggxn[:, d, :],
                                            gxk[:, d, 2, :], op=ALU.add)

            # h0 (zeros) from the warmup collective; the scalar queue's
            # next consumer (fc xft activation) starts later than the
            # warmup finishes, so this wait costs nothing
            nc.scalar.dma_start(hall[:, :, 0:1, 0:1].bitcast(f32),
                                ccz_o[:])

            for oc in range(4):
                if oc == 0:
                    fcm, fcw = fcm0, fcw0
                else:
                    fcm, fcw = fw_tiles()
                    nc.sync.dma_start(fcm[:], fcm_d[:, :, :, oc, :])
                    nc.sync.dma_start(fcw[:], fcw_d[:, :, :, :, oc, :])
                for rg in range(4):
                    ps = psum_aux([128, 8, 64])
                    first = True
                    for ch in range(2):
                        for dy in range(3):
                            for dx in range(3):
                                nc.tensor.matmul(
                                    ps[:],
                                    fcm[:, ch, 3 * dy + dx, :],
                                    xmel[:, ch, rg * 8 + dy:
                                         rg * 8 + dy + 8, dx:dx + 64],
                                    start=first, stop=False)
                                first = False
                    for ch in range(2):
                        for dx in range(3):
                            last = (ch == 1 and dx == 2)
                            if rg == 0:
                                nc.tensor.matmul(
                                    ps[:, 0:1, :],
                                    fcw[:, ch, 1, dx, :],
                                    xwr8[:, ch, 0:1, dx:dx + 64],
                                    start=False, stop=False)
                                nc.tensor.matmul(
                                    ps[:, 1:8, :],
                                    fcw[:, ch, 0, dx, :],
                                    xwr8[:, ch, 0:7, dx:dx + 64],
                                    start=False, stop=last)
                            elif rg == 3:
                                nc.tensor.matmul(
                                    ps[:, 0:7, :],
                                    fcw[:, ch, 0, dx, :],
                                    xwr8[:, ch, 0:7, dx:dx + 64],
                                    start=False, stop=False)
                                nc.tensor.matmul(
                                    ps[:, 7:8, :],
                                    fcw[:, ch, 2, dx, :],
                                    xwr8[:, ch, 0:1, dx:dx + 64],
                                    start=False, stop=last)
                            else:
                                nc.tensor.matmul(
                                    ps[:],
                                    fcw[:, ch, 0, dx, :],
                                    xwr8[:, ch, :, dx:dx + 64],
                                    start=False, stop=last)
                    xft = sc.tile([128, 8, 64], f32r, tag="xf", bufs=3)
                    nc.scalar.activation(xft[:], ps[:], AF.Relu,
                                         bias=fb4[:, oc:oc + 1])
                    nc.vector.tensor_reduce(
                        featp[:, oc, rg * 8:rg * 8 + 8], xft[:],
                        axis=AX.X, op=ALU.add)
                # ship this oc-chunk into the staging buffer; fire an
                # AllReduce for each half (the CC engine serializes
                # collectives, so two 32KB ARs beat four 16KB ones).
                half = oc // 2
                nc.sync.dma_start(ccins[half][oc % 2], featp[:, oc, :])
                if oc % 2 == 1:
                    nc.gpsimd.collective_compute(
                        "AllReduce", ALU.add,
                        replica_groups=[list(range(NCORES))],
                        ins=[ccins[half].opt()], outs=[ccouts[half].opt()])
                    # results land directly in the f32r gx operand tile
                    # (two queues so the dmas run in parallel)
                    nc.gpsimd.dma_start(
                        ftr[:, 2 * half, :].bitcast(f32), ccouts[half][0])
                    nc.scalar.dma_start(
                        ftr[:, 2 * half + 1, :].bitcast(f32),
                        ccouts[half][1])
                # lagged gx for the first half while the second computes
                if oc == 3:
                    emit_gx(0)
                    emit_gx(1)
            emit_gx(2)
            emit_gx(3)

            # ============== GRU (replicated) =============================
            # Per (step, dir): 3 gate matmuls + a k=1 matmul injecting
            # bhh_n into the n-gate psum.  The n gate then computes as
            # tanh(r*psum + gx_n) in ONE scalar instr (scale=r), so the
            # critical chain is matmul -> sig_z' -> sig_r -> tanh -> h'
            # with r/z' in separate tiles (no false deps).
            for s in range(32):
                pss = []
                for d in range(2):
                    ps = psum_grz([128, 2, 2])
                    psn = psum_gn([128, 2])
                    nc.tensor.matmul(ps[:, 1, :], whhT[:, d, 1, :],
                                     hall[:, d, s, :],
                                     start=True, stop=True)
                    nc.tensor.matmul(ps[:, 0, :], whhT[:, d, 0, :],
                                     hall[:, d, s, :],
                                     start=True, stop=True)
                    # n-gate: bhh_n injected via k=1 matmul, then whh_n*h
                    # accumulates; only the tanh reads this tile, so the
                    # 2-matmul group fences nothing else.
                    nc.tensor.matmul(psn[:], bhhnT[0:1, d, :], one2[:],
                                     start=True, stop=False)
                    nc.tensor.matmul(psn[:], whhT[:, d, 2, :],
                                     hall[:, d, s, :],
                                     start=False, stop=True)
                    pss.append((ps, psn))
                for d in range(2):
                    t = s if d == 0 else 31 - s
                    ps, psn = pss[d]
                    zt = sc.tile([128, 1], f32, tag="zt", bufs=8)
                    nc.scalar.activation(zt[:], ps[:, 1, 0:1],
                                         AF.Sigmoid, scale=-1.0,
                                         bias=ggxrz[:, d, t, 1:2])
                    rt = sc.tile([128, 1], f32, tag="rt", bufs=8)
                    nc.scalar.activation(rt[:], ps[:, 0, 0:1],
                                         AF.Sigmoid,
                                         bias=ggxrz[:, d, t, 0:1])
                    nt = sc.tile([128, 1], f32, tag="nt", bufs=8)
                    nc.scalar.activation(nt[:], psn[:, 0:1], AF.Tanh,
                                         scale=rt[:, 0:1],
                                         bias=ggxn[:, d, t:t + 1])
                    # zt holds z' = 1-z.  hmn = h*z' - h (off critical
                    # path); h' = n*z' - hmn = (1-z)*n + z*h
                    hmn = sc.tile([128, 1], f32, tag="hmn", bufs=8)
                    nc.vector.scalar_tensor_tensor(
                        hmn[:], hall[:, d, s, 0:1], zt[:, 0:1],
                        hall[:, d, s, 0:1], op0=ALU.mult, op1=ALU.subtract)
                    nc.vector.scalar_tensor_tensor(
                        hall[:, d, s + 1, 0:1], nt[:], zt[:, 0:1], hmn[:],
                        op0=ALU.mult, op1=ALU.subtract)

            p_fw.release()
            nc.vector.tensor_reduce(hsum[:], hall[:, :, 1:33, 0],
                                    axis=AX.X, op=ALU.add)
            psc = psum_aux([5, 1])
            for d in range(2):
                nc.tensor.matmul(psc[:], clsT[:, d, :], hsum[:, d:d + 1],
                                 start=(d == 0), stop=(d == 1))
            lgt = sc.tile([5, 1], f32, tag="lgt")
            nc.scalar.activation(lgt[:], psc[:], AF.Identity,
                                 bias=clsb5[:, 0:1])
            nc.sync.dma_start(out_d[0:1, :].rearrange("a p -> p a"), lgt[:])
            p_gru.release()
            p_m2.release()

    nc.compile()
    return nc


def _prep_inputs(inputs):
    """Build the 8 per-core input maps from the full model inputs."""
    f = np.float32
    wave = np.asarray(inputs["waveform"], f).reshape(16000)
    logmel = np.asarray(inputs["logmel"], f).reshape(64, 1024)

    wp = np.zeros(16015, f)
    wp[3:16003] = wave
    wP = np.ascontiguousarray(wp.reshape(3203, 5).T)   # [5, 3203]

    R = _resize_matrix(50, 512)
    Rp = np.zeros((50, 514), f)
    Rp[:, 1:513] = R

    lmp = np.pad(logmel, ((1, 1), (4, 4)))

    w1m = np.ascontiguousarray(
        np.asarray(inputs["mc1"], f).reshape(64, 9).T)
    s1 = np.asarray(inputs["bn1g"], f) / np.sqrt(
        np.asarray(inputs["bn1v"], f) + 1e-5)
    b1 = (np.asarray(inputs["mb1"], f) - np.asarray(inputs["bn1m"], f)) * s1 \
        + np.asarray(inputs["bn1b"], f)
    mc2 = np.asarray(inputs["mc2"], f)              # [128, 64, 3, 3]
    w2m = np.zeros((128, 6, 128), f)
    for dx in range(3):
        w2m[0:64, dx, :] = mc2[:, :, 0, dx].T
        w2m[64:128, dx, :] = mc2[:, :, 1, dx].T
        w2m[0:64, 3 + dx, :] = mc2[:, :, 2, dx].T
    s2 = np.asarray(inputs["bn2g"], f) / np.sqrt(
        np.asarray(inputs["bn2v"], f) + 1e-5)
    b2 = (np.asarray(inputs["mb2"], f) - np.asarray(inputs["bn2m"], f)) * s2 \
        + np.asarray(inputs["bn2b"], f)
    w3m = np.ascontiguousarray(
        np.asarray(inputs["mc3"], f).reshape(256, 128).T.reshape(128, 2, 128))
    b3 = np.ascontiguousarray(
        np.asarray(inputs["mb3"], f).reshape(2, 128).T)

    fc = np.asarray(inputs["fc"], f)                   # [512,512,3,3]
    fcmel = fc[:, 256:, :, :]
    fcm = np.ascontiguousarray(
        fcmel.reshape(4, 128, 2, 128, 9).transpose(3, 2, 4, 0, 1))
    fcwave = fc[:, :256, :, :]
    wsum = np.stack([
        fcwave.sum(axis=2),
        fcwave[:, :, 1:, :].sum(axis=2),
        fcwave[:, :, :2, :].sum(axis=2),
    ], axis=2)                              # [512, 256, 3var, 3dx]
    fcw = np.ascontiguousarray(
        wsum.reshape(4, 128, 2, 128, 3, 3).transpose(3, 2, 4, 5, 0, 1))
    fb4 = np.ascontiguousarray(
        np.asarray(inputs["fb"], f).reshape(4, 128).T)

    wc1 = np.asarray(inputs["wc1"], f).reshape(64, 11)
    w1w = np.zeros((5, 3, 64), f)
    for tap in range(11):
        w1w[tap % 5, tap // 5, :] = wc1[:, tap]
    w2w = np.ascontiguousarray(
        np.asarray(inputs["wc2"], f).reshape(128, 64, 5)
        .transpose(1, 2, 0))
    w3w = np.ascontiguousarray(
        np.asarray(inputs["wc3"], f).reshape(256, 128, 5)
        .transpose(1, 2, 0).reshape(128, 5, 2, 128))
    wb3w = np.ascontiguousarray(
        np.asarray(inputs["wb3"], f).reshape(2, 128).T)
    w4wT = np.ascontiguousarray(
        np.asarray(inputs["wc4"], f).reshape(256, 256, 5)
        .transpose(1, 2, 0).reshape(2, 128, 5, 256).transpose(1, 0, 2, 3))
    wb4row = np.asarray(inputs["wb4"], f).reshape(1, 256)
    one50 = np.ones((1, 50), f)

    def gru_prep(d):
        wih = np.asarray(inputs[f"wih_{d}"], f) / 512.0
        whh = np.asarray(inputs[f"whh_{d}"], f)
        bih = np.asarray(inputs[f"bih_{d}"], f)
        bhh = np.asarray(inputs[f"bhh_{d}"], f)
        wihT = np.ascontiguousarray(
            wih.reshape(3, 128, 4, 128).transpose(3, 0, 2, 1))
        whhT = np.ascontiguousarray(
            whh.reshape(3, 128, 128).transpose(2, 0, 1))
        brz = (bih + bhh)[:256].reshape(2, 128).T
        return wihT, whhT, brz, bih[256:], bhh[256:]

    wihT_f, whhT_f, brz_f, bn_f, bhn_f = gru_prep("f")
    wihT_b, whhT_b, brz_b, bn_b, bhn_b = gru_prep("b")
    wihT = np.ascontiguousarray(np.stack([wihT_f, wihT_b], axis=1))
    whhT = np.ascontiguousarray(np.stack([whhT_f, whhT_b], axis=1))
    brz = np.ascontiguousarray(np.stack([brz_f, brz_b], axis=1))
    brz[:, :, 1] *= -1.0
    bng = np.ascontiguousarray(np.stack([bn_f, bn_b], axis=1))
    bhhn = np.ascontiguousarray(np.stack([bhn_f, bhn_b], axis=1))
    bhhnT = np.ascontiguousarray(bhhn.T.reshape(1, 2, 128))
    clsW = np.asarray(inputs["clsW"], f) / 32.0
    clsT = np.ascontiguousarray(
        clsW.reshape(5, 2, 128).transpose(2, 1, 0))
    clsb5 = np.asarray(inputs["clsb"], f).reshape(5, 1)

    shared = dict(
        wP=wP, w1m=w1m, bn1s=s1.reshape(64, 1), bn1b=b1.reshape(64, 1),
        w2m=w2m, bn2s=s2.reshape(128, 1), bn2b=b2.reshape(128, 1),
        w3m=w3m, b3=b3, fcm=fcm, fcw=fcw, fb4=fb4,
        w1w=w1w, wb1=np.asarray(inputs["wb1"], f).reshape(64, 1),
        w2w=w2w, wb2=np.asarray(inputs["wb2"], f).reshape(128, 1),
        w3w=w3w, wb3w=wb3w, w4wT=w4wT, wb4row=wb4row, one50=one50,
        wihT=wihT, brz=brz, bng=bng, bhhnT=bhhnT, whhT=whhT,
        one2=np.ones((1, 2), f),
        clsT=clsT, clsb5=clsb5, zed=np.zeros((128, 2), f),
    )
    in_maps = []
    for i in range(NCORES):
        m = dict(shared)
        lms = lmp[:, 128 * i:128 * i + 136]
        lmI = np.empty((9, 8576), f)
        for dy in range(3):
            for dx in range(3):
                lmI[3 * dy + dx] = lms[dy:dy + 64, dx:dx + 134].reshape(-1)
        m["lmI"] = lmI
        m["Ri"] = np.ascontiguousarray(Rp[:, 64 * i:64 * i + 66])
        mk = np.ones((128, 2), f)
        if i == 0:
            mk[:, 0] = 0.0
        if i == NCORES - 1:
            mk[:, 1] = 0.0
        m["msk"] = mk
        in_maps.append(m)
    return in_maps


def kernel(**inputs):
    global LAST_RESULTS
    _ensure_concourse()
    from concourse import bass_utils

    if "nc" not in _CACHE:
        _CACHE["nc"] = _build()
    nc = _CACHE["nc"]
    in_maps = _prep_inputs(inputs)
    res = bass_utils.run_bass_kernel_spmd(
        nc, in_maps, core_ids=list(range(NCORES)))
    LAST_RESULTS = res
    return res.results[0]["out"]


if __name__ == "__main__":
    _ensure_concourse()
    _build()
    print("build + compile OK")
